# revision 4
# baseline (speedup 1.0000x reference)
"""Distributed multi-head attention kernel for one TRN2 chip (8 NeuronCores).

Problem: x[4, 2048, 1024] -> qkv Linear(1024, 3072, bias=False) -> 16-head
softmax attention -> proj Linear(1024, 1024) + bias.

Sharding: tensor-parallel over heads. Core c owns heads {2c, 2c+1} (128 of the
1024 qkv feature dims). Each core computes Q/K/V for its head pair over the
full sequence, runs attention per (batch, head), then the chip reshards with
two AllToAlls per batch (one per 1024-token half) so core c ends up with the
full 1024 attention features for tokens {half*1024 + 128c ..+128} of every
batch. Each core applies the full W_proj to its token slices and the host
concatenates the shards.

Key engine-level structure (what makes this fast):
 - QKV matmuls run in fp8e4m3 with DoubleRow perf mode (2 contraction
   channels per PE cell): half the PE cycles of bf16.
 - Scores S^T = K Q^T per head are K=64 matmuls; the two heads use PE row
   tiles (0,0)/(64,0) and run concurrently -> full PE utilization.
 - exp on the ScalarEngine (the true bottleneck: ~1 el/cycle/lane @1.2GHz,
   ~294us/core total). Emission interleaves scores matmul pairs with PV/QKV/
   proj matmul chunks so the ACT engine is never starved while the PE works.
 - PV is restructured: stationary = P^T tile [128 ktok, 128 q] (fp8, from the
   exp output), moving = [V | 1] [128 ktok, 65] fp8. Out = [128 q, 65] f32
   accumulated over 16 k-tiles. M=128 means full PE columns (the old layout
   wasted half the array on M=65), and column 64 of the output is the softmax
   denominator as a per-partition scalar, so the divide is a DVE
   reciprocal + tensor_scalar_mul, no partition broadcast needed.
 - O comes out in natural [token, feature] layout; the [feat, token] layout
   needed for the A2A/proj is produced by hardware XBAR DMA transposes
   (dma_start_transpose, ~435GB/s SBUF->SBUF).
 - Each batch reshards with two half-token AllToAlls so the last batch's
   projection overlaps the final collective (short tail).
"""

import os
import sys

import numpy as np

for _p in ("/opt/trn_rl_repo", "/root/.axon_site/_ro/trn_rl_repo"):
    if os.path.isdir(_p) and _p not in sys.path:
        sys.path.append(_p)

import ml_dtypes  # noqa: E402

B, N, C = 4, 2048, 1024
NUM_HEADS = 16
HEAD_DIM = C // NUM_HEADS  # 64
SCALE = HEAD_DIM**-0.5
NCORES = 8
P = 128  # SBUF partitions
QC = 512  # q-chunk (matmul free dim / PSUM bank)
NCC2 = 4  # 256-channel DoubleRow contraction chunks

BF16 = ml_dtypes.bfloat16
FP8 = ml_dtypes.float8_e4m3


def build_attention_nc(NB: int = B, NQ: int = N, CH: int = C):
    """Build + compile the SPMD graph. NB batches of NQ tokens, CH channels.

    Every core runs the same graph; per-core behavior differs only through the
    per-core input shards (wq/wk/wv slices) and the AllToAlls.
    """
    import concourse.bass as bass
    import concourse.mybir as mybir
    import concourse.tile as tile
    from concourse import bacc

    f32 = mybir.dt.float32
    bf16 = mybir.dt.bfloat16
    fp8 = mybir.dt.float8e4
    DR = mybir.MatmulPerfMode.DoubleRow

    n_qc = NQ // QC  # q chunks per batch (4)
    n_kt = NQ // P  # k tiles per batch (16)
    n_cc = CH // P  # bf16 contraction chunks (8, for proj)
    NQW = QC // P  # PV q-windows per q-chunk (4)
    TPB = NQ // NCORES  # tokens per core per batch after reshard (256)
    HTOK = NQ // 2  # tokens per A2A half (1024)

    nc = bacc.Bacc("TRN2", target_bir_lowering=False, debug=False,
                   num_devices=NCORES)

    x8 = nc.dram_tensor("x8", [NCC2, P, 2, NB * NQ], fp8,
                        kind="ExternalInput").ap()
    wq = nc.dram_tensor("wq", [NCC2, P, 2, P], fp8, kind="ExternalInput").ap()
    wk = nc.dram_tensor("wk", [NCC2, P, 2, P], fp8, kind="ExternalInput").ap()
    wv = nc.dram_tensor("wv", [NCC2, P, 2, P], fp8, kind="ExternalInput").ap()
    wp = nc.dram_tensor("wp", [CH, CH], bf16, kind="ExternalInput").ap()
    bp = nc.dram_tensor("bp", [1, CH], f32, kind="ExternalInput").ap()
    out = nc.dram_tensor("out", [NB * TPB, CH], f32, kind="ExternalOutput").ap()

    from contextlib import ExitStack

    with tile.TileContext(nc) as tc, ExitStack() as ctx:
        const = ctx.enter_context(tc.tile_pool(name="const", bufs=1))
        xt_pool = ctx.enter_context(tc.tile_pool(name="xt", bufs=2 * NCC2 + 1))
        qkv_pool = ctx.enter_context(tc.tile_pool(name="qkv", bufs=2))
        pt_pool = ctx.enter_context(tc.tile_pool(name="pt", bufs=3))
        onat_pool = ctx.enter_context(tc.tile_pool(name="onat", bufs=8))
        ot_pool = ctx.enter_context(tc.tile_pool(name="ot", bufs=2))
        div_pool = ctx.enter_context(tc.tile_pool(name="div", bufs=4))
        at_pool = ctx.enter_context(tc.tile_pool(name="at", bufs=3 * n_cc))
        y_pool = ctx.enter_context(tc.tile_pool(name="y", bufs=3))
        dram = ctx.enter_context(tc.tile_pool(name="dram", bufs=1, space="DRAM"))
        # PSUM budget (8 banks): ST 2x2 + mm 3 + pv 1
        ps_st = ctx.enter_context(tc.tile_pool(name="ps_st", bufs=2, space="PSUM"))
        ps_mm = ctx.enter_context(tc.tile_pool(name="ps_mm", bufs=3, space="PSUM"))
        ps_pv = ctx.enter_context(tc.tile_pool(name="ps_pv", bufs=1, space="PSUM"))

        # --- resident weights ---
        wq_sb = const.tile([P, NCC2, 2, P], fp8, tag="wq")
        wk_sb = const.tile([P, NCC2, 2, P], fp8, tag="wk")
        wv_sb = const.tile([P, NCC2, 2, P], fp8, tag="wv")
        nc.sync.dma_start(wq_sb[:], wq.rearrange("cc ki ko m -> ki cc ko m"))
        nc.sync.dma_start(wk_sb[:], wk.rearrange("cc ki ko m -> ki cc ko m"))
        nc.sync.dma_start(wv_sb[:], wv.rearrange("cc ki ko m -> ki cc ko m"))
        wp_sb = const.tile([P, n_cc, CH], bf16, tag="wp")
        bias_row = const.tile([1, CH], f32, tag="bias_row")
        bias_sb = const.tile([P, CH], f32, tag="bias")

        a2a_in = [[None, None] for _ in range(NB)]
        a2a_out = [[None, None] for _ in range(NB)]
        for b in range(NB):
            for hf in range(2):
                a2a_in[b][hf] = dram.tile([NCORES * P, P], bf16,
                                          tag=f"a2a_in{b}_{hf}",
                                          name=f"a2a_in{b}_{hf}")
                a2a_out[b][hf] = dram.tile([NCORES * P, P], bf16,
                                           tag=f"a2a_out{b}_{hf}",
                                           name=f"a2a_out{b}_{hf}")

        def emit_xt(b, split=1):
            xts = [xt_pool.tile([P, 2, NQ], fp8, tag="xt", name="xt_tile")
                   for _ in range(NCC2)]
            hw = NQ // split
            for hv in range(split):
                for cc in range(NCC2):
                    nc.sync.dma_start(
                        xts[cc][:, :, hv * hw:(hv + 1) * hw],
                        x8[cc, :, :, b * NQ + hv * hw:b * NQ + (hv + 1) * hw])
            return xts

        # ---- generator-based PE work, pumped between scores/exp steps ----

        def gen_qkv_qk(xts, dst_pair):
            """QT/KT [128 head-dims, NQ tokens] via fp8 DoubleRow matmuls."""
            qt_sb, kt_sb = dst_pair
            for qc in range(0, n_qc, 2):
                for w_sb, dst in ((wq_sb, qt_sb), (wk_sb, kt_sb)):
                    qsa = slice(qc * QC, (qc + 1) * QC)
                    qsb = slice((qc + 1) * QC, (qc + 2) * QC)
                    psa = ps_mm.tile([P, QC], f32, tag="mm", name="ps_a")
                    psb = ps_mm.tile([P, QC], f32, tag="mm", name="ps_b")
                    for cc in range(NCC2):
                        nc.tensor.matmul(psa[:], w_sb[:, cc], xts[cc][:, :, qsa],
                                         perf_mode=DR,
                                         start=(cc == 0), stop=(cc == NCC2 - 1))
                        yield
                        nc.tensor.matmul(psb[:], w_sb[:, cc], xts[cc][:, :, qsb],
                                         perf_mode=DR,
                                         start=(cc == 0), stop=(cc == NCC2 - 1))
                        yield
                    nc.vector.tensor_copy(dst[:, qsa], psa[:])
                    nc.vector.tensor_copy(dst[:, qsb], psb[:])

        def gen_qkv_v(xts, v_sb):
            """V in natural [tok, head-dim] fp8 layout + ones column per head."""
            nc.vector.memset(v_sb[:, :, :, 64:65], 1.0)
            for tt in range(n_kt):
                ts_ = slice(tt * P, (tt + 1) * P)
                vps = ps_mm.tile([P, P], f32, tag="mm", name="vps")
                for cc in range(NCC2):
                    nc.tensor.matmul(vps[:], xts[cc][:, :, ts_], wv_sb[:, cc],
                                     perf_mode=DR,
                                     start=(cc == 0), stop=(cc == NCC2 - 1))
                yield
                nc.vector.tensor_copy(
                    v_sb[:, tt, :, 0:64], vps.rearrange("p (g c) -> p g c", g=2))

        def gen_pv(qc, pt_t, v_sb, ot_sb):
            """O[q, d] = P V per head; col 64 = softmax denominator.

            Stationary = P^T tiles [128 ktok, 128 q] fp8, moving = [V|1]
            [128 ktok, 65] fp8; full-M matmuls, N=65. Divide on DVE with the
            denominator as a per-partition scalar, then XBAR-transpose each
            [q, feat] tile into ot_sb [feat, q]."""
            onats = [onat_pool.tile([P, P], bf16, tag="onat", name="onat_t")
                     for _ in range(NQW)]
            for h in range(2):
                apv = ps_pv.tile([P, NQW, 65], f32, tag="pv", name="apv_t")
                for kt in range(n_kt):
                    for qw in range(NQW):
                        nc.tensor.matmul(
                            apv[:, qw, :],
                            pt_t[:, 2 * kt + h, qw * P:(qw + 1) * P],
                            v_sb[:, kt, h, :],
                            start=(kt == 0), stop=(kt == n_kt - 1))
                    if kt % 4 == 3:
                        yield
                for qw in range(NQW):
                    rec = div_pool.tile([P, 1], f32, tag="rec", name="rec_t")
                    nc.vector.reciprocal(rec[:], apv[:, qw, 64:65])
                    nc.vector.tensor_scalar_mul(
                        onats[qw][:, 64 * h:64 * (h + 1)],
                        apv[:, qw, 0:64], rec[:])
                yield
            for qw in range(NQW):
                nc.sync.dma_start_transpose(
                    ot_sb[:, qc * QC + qw * P:qc * QC + (qw + 1) * P],
                    onats[qw][:])

        def gen_proj(b, hf, ats):
            """W_proj + bias for this core's 128 tokens of (batch b, half hf)."""
            for oc in range(CH // QC):
                ocs = slice(oc * QC, (oc + 1) * QC)
                yps = ps_mm.tile([P, QC], f32, tag="mm", name="yps_t")
                for cc in range(n_cc):
                    nc.tensor.matmul(yps[:], ats[cc][:], wp_sb[:, cc, ocs],
                                     start=(cc == 0), stop=(cc == n_cc - 1))
                    if cc % 4 == 3:
                        yield
                y_sb = y_pool.tile([P, QC], f32, tag="y", name="y_tile")
                nc.vector.tensor_add(y_sb[:], yps[:], bias_sb[:, ocs])
                nc.sync.dma_start(
                    out[b * TPB + hf * P:b * TPB + (hf + 1) * P, ocs], y_sb[:])

        # ---- work queue pump ----
        # Background PE work (PV/QKV/proj matmul chains) is emitted through
        # generators pumped between scores/exp steps, so the TensorEngine has
        # dense work while the ScalarEngine grinds through exp (the
        # bottleneck), instead of the engines ping-ponging phase by phase.
        from collections import deque
        work = deque()

        def pump(n):
            for _ in range(n):
                while work:
                    try:
                        next(work[0])
                        break
                    except StopIteration:
                        work.popleft()
                if not work:
                    break

        def finish(g):
            """Fully emit generator g (it may be anywhere in the queue)."""
            if g is None:
                return
            for _ in g:
                pass

        def drain():
            while work:
                try:
                    next(work[0])
                except StopIteration:
                    work.popleft()

        def emit_scores(qc, qt_sb, kt_sb, pump_per_kt=2):
            """S^T = K Q^T row-tiled head pair -> exp -> pt (fp8).

            Between score-MM pairs, pump ~2 chunks of background PE work so
            the TensorEngine fills the exp-wait gaps instead of idling."""
            qs = slice(qc * QC, (qc + 1) * QC)
            pt_t = pt_pool.tile([P, 2 * n_kt, QC], fp8, tag="pt",
                                name="pt_tile")
            for kt in range(n_kt):
                ks = slice(kt * P, (kt + 1) * P)
                st = ps_st.tile([P, 2, QC], f32, tag="st", name="st_tile")
                for h in range(2):
                    hs = slice(64 * h, 64 * (h + 1))
                    nc.tensor.matmul(st[:, h, :], kt_sb[hs, ks],
                                     qt_sb[hs, qs])
                nc.scalar.activation(pt_t[:, 2 * kt:2 * kt + 2, :], st[:],
                                     mybir.ActivationFunctionType.Exp,
                                     scale=SCALE)
                pump(pump_per_kt)
            return pt_t

        def emit_a2a(b, hf, ot_sb):
            nc.gpsimd.dma_start(
                a2a_in[b][hf].rearrange("(j p) t -> p j t", p=P),
                ot_sb[:, hf * HTOK:(hf + 1) * HTOK].rearrange(
                    "p (j t) -> p j t", j=NCORES))
            nc.gpsimd.collective_compute(
                "AllToAll", mybir.AluOpType.bypass,
                replica_groups=[list(range(NCORES))],
                ins=[a2a_in[b][hf][:].opt()], outs=[a2a_out[b][hf][:].opt()])

        def emit_proj_loads(b, hf):
            ats = []
            for cc in range(n_cc):
                at = at_pool.tile([P, P], bf16, tag="at", name="at_tile")
                nc.sync.dma_start(at[:], a2a_out[b][hf][cc * P:(cc + 1) * P, :])
                ats.append(at)
            return ats

        # ---- main program ----
        # Batch loop is software-pipelined one stage deep: batch b-1's last
        # PV chunk and second AllToAll are carried into batch b's scores
        # region, so the PE keeps working through the exp of the next batch
        # and the ACT engine never waits on a batch boundary.
        xts = emit_xt(0, split=4)
        qt_sb = qkv_pool.tile([P, NQ], bf16, tag="qt")
        kt_sb = qkv_pool.tile([P, NQ], bf16, tag="kt")
        v_sb = qkv_pool.tile([P, n_kt, 2, 65], fp8, tag="v")
        # batch 0 QK runs inline (nothing else to do yet); V via the queue so
        # it interleaves with the first scores/exp steps
        for _ in gen_qkv_qk(xts, (qt_sb, kt_sb)):
            pass
        g_v = gen_qkv_v(xts, v_sb)
        work.append(g_v)
        # W_proj + bias aren't needed until the first projection
        nc.sync.dma_start(wp_sb[:], wp.rearrange("(cc p) m -> p cc m", p=P))
        nc.sync.dma_start(bias_row[:], bp[:, :])
        nc.gpsimd.partition_broadcast(bias_sb[:], bias_row[:])

        g_qk = g_pv_carry = None
        prev_ot = None
        for b in range(NB):
            # carried-in generators that this batch's scores emission reads
            # from must be fully emitted first (reads may never be emitted
            # before their writers)
            finish(g_qk)
            finish(g_v)
            if b + 1 < NB:
                next_xts = emit_xt(b + 1)
                nqt = qkv_pool.tile([P, NQ], bf16, tag="qt", name="nqt")
                nkt = qkv_pool.tile([P, NQ], bf16, tag="kt", name="nkt")
                nv = qkv_pool.tile([P, n_kt, 2, 65], fp8, tag="v", name="nv")
            if b > 0:
                ats0 = emit_proj_loads(b - 1, 0)
                work.append(gen_proj(b - 1, 0, ats0))
            if b + 1 < NB:
                g_qk = gen_qkv_qk(next_xts, (nqt, nkt))
                g_v = gen_qkv_v(next_xts, nv)
                work.append(g_qk)
                work.append(g_v)
            else:
                g_qk = g_v = None

            ot_sb = ot_pool.tile([P, NQ], bf16, tag="ot")
            pt0 = emit_scores(0, qt_sb, kt_sb)
            if b > 0:
                # batch b-1's tail: its last PV ran during exp(b, 0) above
                finish(g_pv_carry)
                emit_a2a(b - 1, 1, prev_ot)
                ats1 = emit_proj_loads(b - 1, 1)
                work.append(gen_proj(b - 1, 1, ats1))
            pt1 = emit_scores(1, qt_sb, kt_sb)
            g_pv0 = gen_pv(0, pt0, v_sb, ot_sb)
            work.append(g_pv0)
            pt2 = emit_scores(2, qt_sb, kt_sb)
            g_pv1 = gen_pv(1, pt1, v_sb, ot_sb)
            work.append(g_pv1)
            pt3 = emit_scores(3, qt_sb, kt_sb)
            finish(g_pv0)
            finish(g_pv1)
            emit_a2a(b, 0, ot_sb)
            work.append(gen_pv(2, pt2, v_sb, ot_sb))
            g_pv_carry = gen_pv(3, pt3, v_sb, ot_sb)
            work.append(g_pv_carry)
            prev_ot = ot_sb
            if b + 1 < NB:
                qt_sb, kt_sb, v_sb = nqt, nkt, nv
                xts = next_xts

        # final tail: last PV, last collective, last projections
        drain()
        emit_a2a(NB - 1, 1, prev_ot)
        ats0 = emit_proj_loads(NB - 1, 0)
        for _ in gen_proj(NB - 1, 0, ats0):
            pass
        ats1 = emit_proj_loads(NB - 1, 1)
        for _ in gen_proj(NB - 1, 1, ats1):
            pass

    nc.compile()
    return nc


def make_in_maps(x, W_qkv, W_proj, b_proj, NB=B, NQ=N, CH=C):
    """Shard the full inputs into one input map per core."""
    T = NB * NQ
    # x -> [NCC2, ki, ko, T] fp8 (channel c = cc2*256 + ko*128 + ki)
    xT = np.ascontiguousarray(x.reshape(T, CH).T)  # [CH, T]
    x8 = np.ascontiguousarray(
        xT.reshape(NCC2, 2, P, T).transpose(0, 2, 1, 3)).astype(FP8)
    wp = np.ascontiguousarray(W_proj).astype(BF16)
    bp = np.ascontiguousarray(b_proj[None, :]).astype(np.float32)

    def w8(mat):  # [CH, 128] -> [NCC2, ki, ko, 128] fp8
        return np.ascontiguousarray(
            mat.reshape(NCC2, 2, P, P).transpose(0, 2, 1, 3)).astype(FP8)

    in_maps = []
    for c in range(NCORES):
        cs = slice(P * c, P * (c + 1))
        in_maps.append({
            "x8": x8,
            "wq": w8(W_qkv[:, cs]),
            "wk": w8(W_qkv[:, CH:][:, cs]),
            "wv": w8(W_qkv[:, 2 * CH:][:, cs]),
            "wp": wp,
            "bp": bp,
        })
    return in_maps


def assemble_output(results, NB=B, NQ=N, CH=C):
    """Concatenate the per-core token shards into the full output.

    Core c's out rows [b*256 + hf*128 .. +128] hold tokens
    [hf*1024 + 128c .. +128] of batch b."""
    TPB = NQ // NCORES
    full = np.empty((NB, NQ, CH), dtype=np.float32)
    for c in range(NCORES):
        y = np.asarray(results[c]["out"], dtype=np.float32)
        for b in range(NB):
            for hf in range(2):
                full[b, hf * (NQ // 2) + P * c:hf * (NQ // 2) + P * (c + 1), :] = \
                    y[b * TPB + hf * P:b * TPB + (hf + 1) * P]
    return full


_compiled_nc = None


def kernel(x, W_qkv, W_proj, b_proj):
    global _compiled_nc
    x = np.asarray(x, dtype=np.float32)
    W_qkv = np.asarray(W_qkv, dtype=np.float32)
    W_proj = np.asarray(W_proj, dtype=np.float32)
    b_proj = np.asarray(b_proj, dtype=np.float32)

    if _compiled_nc is None:
        _compiled_nc = build_attention_nc()

    from concourse.bass_utils import run_bass_kernel_spmd

    in_maps = make_in_maps(x, W_qkv, W_proj, b_proj)
    res = run_bass_kernel_spmd(_compiled_nc, in_maps,
                               core_ids=list(range(NCORES)))
    return assemble_output(res.results)
